# revision 1
# baseline (speedup 1.0000x reference)
"""NT-Xent / SimCLR contrastive loss on 8 Trainium2 NeuronCores (v3).

Math (matches the jax reference):
    z = l2_normalize(concat([emb_i, emb_j]))          # [2B, D] unit rows
    sim = z @ z.T                                     # cosine similarities
    denom_r = sum_{j != r} exp(sim_rj / T)
    pos_r   = z_r . z_{(r+B) mod 2B}                  # the positive pair
    loss = mean_r( log(denom_r) - pos_r / T )

Sharding: the 2B=8192 rows are data-parallel over 8 cores. Each core
receives the representation matrix ROTATED by -c*1024 rows, so its slab
is always local rows 0..1023 (one SPMD program for all cores).

v3 exploits the symmetry of exp(sim/T): each core computes only local
columns 0..5119 (blocks 0..4 of 8). The row-sum contribution of the
missing blocks 5..7 equals COLUMN sums of blocks 3,2,1 on cores
c+5,c+6,c+7; those [1024]-vectors are tiny and exchanged on the host,
which assembles denominators and takes the log (the "all-reduce" step).

Per-core pipeline:
  - 5 x 1MiB strided loads striped over the 16 SWDGE queues
  - per-group sum-of-squares (DVE square+row-accumulate), rsqrt as
    exp(-0.5*ln(x)) on ACT, normalize+cast to fp8e4 on gpsimd
  - transpose to [D, 5120] fp8 fully on the PE (fp8 identity, 1 cyc/row)
    through a single one-bank PSUM tile with ping-pong slots
  - similarity via fp8 DoubleRow matmuls: one [128x512] output per
    instruction contracts the full D=256 at 0.5 cycles/row
  - ACT computes exp(2*sim) out of PSUM with fused row accumulation;
    exp tiles for blocks 1..3 are kept in fp8 and column-summed with
    ones-DoubleRow matmuls into per-chunk PSUM slots (partition 32*k)
  - exact diagonal e^2 subtracted; positives in fp32; outputs per core:
    row partials [128,8], positives [128,8], column sums [6,512]
"""

import numpy as np
from contextlib import ExitStack

import ml_dtypes
import concourse.bass as bass
import concourse.tile as tile
from concourse import bacc, mybir
from concourse._compat import with_exitstack
from concourse.bass_utils import run_bass_kernel_spmd

B = 4096
D = 256
R = 2 * B
N_CORES = 8
SLAB = R // N_CORES          # 1024 rows per core
INV_T = 2.0
E2 = float(np.exp(2.0))

NCOL = 5 * SLAB              # 5120 columns computed per core
NG = 5                       # load groups of 1024 rows (8 tiles)
NJ = 10                      # rts groups (512 cols each)
M_TILES = SLAB // 128        # 8
GROUPS = [[0, 1, 2], [3, 4, 5], [6, 7], [8, 9]]   # chunk (512-col) groups
CS_CHUNKS = {0: [(2, 0)], 1: [(3, 0), (4, 1), (5, 2)], 2: [(6, 0), (7, 1)]}
N_ACC = 4                    # accum slots per m (one per group)

F32 = mybir.dt.float32
FP8 = mybir.dt.float8e4
NP_FP8 = ml_dtypes.float8_e4m3fn


@with_exitstack
def _loss_kernel(ctx: ExitStack, tc: "tile.TileContext", s1_ap: bass.AP,
                 pos_ap: bass.AP, cs_ap: bass.AP, reps_ap: bass.AP,
                 ident_ap: bass.AP, ones_ap: bass.AP):
    nc = tc.nc
    mult = mybir.AluOpType.mult
    add = mybir.AluOpType.add
    Exp = mybir.ActivationFunctionType.Exp
    Ln = mybir.ActivationFunctionType.Ln
    DR = mybir.MatmulPerfMode.DoubleRow

    xpool = ctx.enter_context(tc.tile_pool(name="x", bufs=NG))
    stats = ctx.enter_context(tc.tile_pool(name="stats", bufs=2))
    scales = ctx.enter_context(tc.tile_pool(name="scales", bufs=NG))
    zpool = ctx.enter_context(tc.tile_pool(name="z8", bufs=6))
    z32pool = ctx.enter_context(tc.tile_pool(name="z32", bufs=2))
    prodpool = ctx.enter_context(tc.tile_pool(name="prod", bufs=2))
    rtpool = ctx.enter_context(tc.tile_pool(name="repsT", bufs=NJ))
    cpool = ctx.enter_context(tc.tile_pool(name="const", bufs=1))
    epool = ctx.enter_context(tc.tile_pool(name="esc", bufs=8))
    accpool = ctx.enter_context(tc.tile_pool(name="acc", bufs=1))
    fpool = ctx.enter_context(tc.tile_pool(name="final", bufs=1))

    # PSUM: 1 bank transpose ping-pong + 2x3 banks matmul + 1 bank colsum
    tppool = ctx.enter_context(tc.tile_pool(name="tp", bufs=1, space="PSUM"))
    mmpool = ctx.enter_context(tc.tile_pool(name="mm", bufs=2, space="PSUM"))
    cspool = ctx.enter_context(tc.tile_pool(name="cs", bufs=1, space="PSUM"))

    ident = cpool.tile([128, 128], FP8, tag="ident")
    nc.sync.dma_start(ident[:], ident_ap[:])
    ones = cpool.tile([128, 2, 16], FP8, tag="ones")
    nc.sync.dma_start(ones[:], ones_ap[:])

    # ---- loads (gpsimd SWDGE, striped over all 16 queues) ---------------
    xg = []
    for g in range(NG):
        xt = xpool.tile([128, 8, D], F32, tag="x", name=f"x{g}")
        src = reps_ap[g * 1024:(g + 1) * 1024, :].rearrange(
            "(t p) d -> p t d", p=128)
        nc.gpsimd.dma_start(xt[:], src)
        xg.append(xt)

    rts = [
        rtpool.tile([128, 2, 4, 128], FP8, tag="repsT", name=f"repsT{j}")
        for j in range(NJ)
    ]
    # fp8 PE transpose writes PSUM with element step 2 -> [..., 128, 2]
    tp = tppool.tile([128, 8, 128, 2], FP8, tag="tp")

    # ---- per-group: ssq -> scale -> normalize(fp8) -> transpose ---------
    scg = []

    def emit_group(g):
        ssq = stats.tile([128, 8], F32, tag="ssq", name=f"ssq{g}")
        for t in range(8):
            junk = prodpool.tile([128, D], F32, tag="prod", name=f"sq{g}_{t}")
            nc.vector.scalar_tensor_tensor(
                out=junk[:], in0=xg[g][:, t, :], scalar=1.0,
                in1=xg[g][:, t, :], op0=mult, op1=mult,
                accum_out=ssq[:, t:t + 1],
            )
        lnv = stats.tile([128, 8], F32, tag="lnv", name=f"lnv{g}")
        sc = scales.tile([128, 8], F32, tag="scale", name=f"sc{g}")
        nc.scalar.activation(lnv[:], ssq[:], Ln)
        nc.scalar.activation(sc[:], lnv[:], Exp, scale=-0.5)
        scg.append(sc)

        z8s = []
        for t in range(8):
            z8 = zpool.tile([128, D], FP8, tag="z8", name=f"z8_{g}_{t}")
            nc.gpsimd.tensor_scalar(
                out=z8[:], in0=xg[g][:, t, :],
                scalar1=sc[:, t:t + 1], scalar2=None, op0=mult,
            )
            z8s.append(z8)
        # transpose the 8 tiles (2 rts groups) on the PE
        for jj in range(2):
            j = 2 * g + jj
            for k in range(2):
                for i in range(4):
                    nc.tensor.transpose(
                        tp[:, 4 * k + i, :, 0:1],
                        z8s[4 * jj + i][:, k * 128:(k + 1) * 128],
                        ident[:],
                    )
                nc.vector.tensor_copy(
                    rts[j][:, k, :, :], tp[:, 4 * k:4 * k + 4, :, 0:1])

    # ---- similarity group: DoubleRow matmuls + fused exp/rowsum ---------
    denacc = accpool.tile([128, M_TILES * N_ACC], F32, tag="denacc")
    esc_live = {}

    def emit_mm_group(gi):
        chunks = GROUPS[gi]
        nch = len(chunks)
        for m in range(M_TILES):
            pt = mmpool.tile([128, 3, 512], F32, tag="mm", name=f"pt{gi}_{m}")
            lhsT = rts[m // 4][:, :, m % 4, :]
            for ci, c in enumerate(chunks):
                nc.tensor.matmul(
                    pt[:, ci, :], lhsT=lhsT, rhs=rts[c][:, :, :, :],
                    perf_mode=DR,
                )
            if m % 2 == 0:
                esc = epool.tile([128, 2, 3, 512], FP8, tag="esc",
                                 name=f"esc{gi}_{m // 2}")
                esc_live[(gi, m // 2)] = esc
            esc = esc_live[(gi, m // 2)]
            idx = m * N_ACC + gi
            nc.scalar.activation(
                esc[:, m % 2, :nch, :], pt[:, :nch, :], Exp, scale=INV_T,
                accum_out=denacc[:, idx:idx + 1],
            )

    def emit_colsums(gi):
        for c, slot in CS_CHUNKS[gi]:
            ci = c - GROUPS[gi][0]
            cs = cspool.tile([128, 512], F32, tag="cs", name=f"cs{gi}_{c}")
            for mp in range(4):
                nc.tensor.matmul(
                    cs[0:16, :],
                    lhsT=ones[:, :, :],
                    rhs=esc_live[(gi, mp)][:, :, ci, :],
                    perf_mode=DR,
                    start=(mp == 0), stop=(mp == 3),
                )
            csb = fpool.tile([128, 512], F32, tag="csb", name=f"csb{gi}_{c}",
                             bufs=2)
            nc.vector.tensor_copy(csb[0:1, :], cs[0:1, :])
            nc.sync.dma_start(cs_ap[c - 2:c - 1, :], csb[0:1, :])

    emit_group(0)
    emit_group(1)
    emit_mm_group(0)
    emit_group(2)
    emit_mm_group(1)
    emit_colsums(0)
    emit_group(3)
    emit_mm_group(2)
    emit_colsums(1)
    emit_group(4)

    # ---- positives (fp32; the -1/T factor rides on the scale) -----------
    posneg = accpool.tile([128, M_TILES], F32, tag="posneg")
    scm2 = scales.tile([128, M_TILES], F32, tag="scm2")
    nc.vector.tensor_scalar(
        out=scm2[:], in0=scg[0][:, 0:M_TILES],
        scalar1=-INV_T, scalar2=None, op0=mult,
    )
    for i in range(M_TILES):
        zb = z32pool.tile([128, D], F32, tag="z32", name=f"zb{i}")
        nc.vector.tensor_scalar(
            out=zb[:], in0=xg[4][:, i, :],
            scalar1=scg[4][:, i:i + 1], scalar2=None, op0=mult,
        )
        prod = prodpool.tile([128, D], F32, tag="prod", name=f"pp{i}")
        nc.vector.scalar_tensor_tensor(
            out=prod[:], in0=xg[0][:, i, :], scalar=scm2[:, i:i + 1],
            in1=zb[:], op0=mult, op1=mult,
            accum_out=posneg[:, i:i + 1],
        )
    nc.sync.dma_start(pos_ap[:], posneg[:])

    emit_mm_group(3)
    emit_colsums(2)

    # ---- finalize: reduce per-group row partials, subtract e^2 ----------
    drow = fpool.tile([128, M_TILES], F32, tag="drow")
    nc.vector.tensor_reduce(
        out=drow[:],
        in_=denacc[:].rearrange("p (m g) -> p m g", g=N_ACC),
        axis=mybir.AxisListType.X,
        op=add,
    )
    s1 = fpool.tile([128, M_TILES], F32, tag="s1")
    nc.vector.tensor_scalar(
        out=s1[:], in0=drow[:], scalar1=-E2, scalar2=None, op0=add,
    )
    nc.sync.dma_start(s1_ap[:], s1[:])


_CACHE = {}


def _get_compiled():
    if "nc" not in _CACHE:
        nc = bacc.Bacc("TRN2", target_bir_lowering=False, debug=False)
        reps_in = nc.dram_tensor("reps", [NCOL, D], F32, kind="ExternalInput")
        ident_t = nc.inline_tensor(np.eye(128, dtype=NP_FP8), name="ident")
        ones_t = nc.inline_tensor(np.ones((128, 2, 16), dtype=NP_FP8),
                                  name="ones")
        s1_out = nc.dram_tensor("s1", [128, M_TILES], F32,
                                kind="ExternalOutput")
        pos_out = nc.dram_tensor("pos", [128, M_TILES], F32,
                                 kind="ExternalOutput")
        cs_out = nc.dram_tensor("csum", [6, 512], F32, kind="ExternalOutput")
        with tile.TileContext(nc) as tc:
            _loss_kernel(tc, s1_out.ap(), pos_out.ap(), cs_out.ap(),
                         reps_in.ap(), ident_t.ap(), ones_t.ap())
        nc.compile()
        _CACHE["nc"] = nc
    return _CACHE["nc"]


def make_in_maps(emb_i: np.ndarray, emb_j: np.ndarray):
    reps = np.concatenate(
        [np.asarray(emb_i, dtype=np.float32),
         np.asarray(emb_j, dtype=np.float32)],
        axis=0,
    )
    return [
        {"reps": np.ascontiguousarray(
            np.roll(reps, -c * SLAB, axis=0)[:NCOL])}
        for c in range(N_CORES)
    ]


def run_spmd(emb_i, emb_j, **kwargs):
    nc = _get_compiled()
    in_maps = make_in_maps(emb_i, emb_j)
    return run_bass_kernel_spmd(nc, in_maps, core_ids=list(range(N_CORES)),
                                **kwargs)


def combine(results) -> np.ndarray:
    """Host-side combine: assemble denominators from the per-core row
    partials + exchanged column sums, take logs, and reduce the loss."""
    s1 = [results[c]["s1"].astype(np.float64).T.reshape(SLAB)
          for c in range(N_CORES)]
    pos = [results[c]["pos"].astype(np.float64).T.reshape(SLAB)
           for c in range(N_CORES)]
    cs = [results[c]["csum"].astype(np.float64) for c in range(N_CORES)]
    total = 0.0
    for c in range(N_CORES):
        c1 = cs[(c + 7) % 8][0:2].reshape(SLAB)
        c2 = cs[(c + 6) % 8][2:4].reshape(SLAB)
        c3 = cs[(c + 5) % 8][4:6].reshape(SLAB)
        den = s1[c] + c1 + c2 + c3
        total += float(np.sum(np.log(den) + pos[c]))
    return np.array(total / R, dtype=np.float32)


def kernel(emb_i: np.ndarray, emb_j: np.ndarray) -> np.ndarray:
    res = run_spmd(emb_i, emb_j)
    return combine(res.results)



# revision 11
# speedup vs baseline: 2.3201x; 2.3201x over previous
"""NT-Xent / SimCLR contrastive loss on 8 Trainium2 NeuronCores (v3).

Math (matches the jax reference):
    z = l2_normalize(concat([emb_i, emb_j]))          # [2B, D] unit rows
    sim = z @ z.T                                     # cosine similarities
    denom_r = sum_{j != r} exp(sim_rj / T)
    pos_r   = z_r . z_{(r+B) mod 2B}                  # the positive pair
    loss = mean_r( log(denom_r) - pos_r / T )

Sharding: the 2B=8192 rows are data-parallel over 8 cores. Each core
receives the representation matrix ROTATED by -c*1024 rows, so its slab
is always local rows 0..1023 (one SPMD program for all cores).

v3 exploits the symmetry of exp(sim/T): each core computes only local
columns 0..5119 (blocks 0..4 of 8). The row-sum contribution of the
missing blocks 5..7 equals COLUMN sums of blocks 3,2,1 on cores
c+5,c+6,c+7; those [1024]-vectors are tiny and exchanged on the host,
which assembles denominators and takes the log (the "all-reduce" step).

Per-core pipeline (v4):
  - 5 x 1MiB strided loads striped over the 16 SWDGE queues
  - per-group sum-of-squares (DVE square+row-accumulate), rsqrt via
    integer-seed + 2 Newton steps entirely on DVE (keeps ACT exp-only,
    one activation table set), normalize+cast to fp8e4 on DVE
  - transpose to [D, 5120] fp8 fully on the PE (fp8 identity, 1 cyc/row)
    through a single one-bank PSUM tile with ping-pong slots
  - similarity via fp8 DoubleRow matmuls: one [128x512] output per
    instruction contracts the full D=256 at 0.5 cycles/row
  - ACT computes exp(2*sim) out of PSUM with fused row accumulation;
    exp tiles for blocks 1..3 are kept in fp8 and column-summed with
    ones-DoubleRow matmuls into per-chunk PSUM slots (partition 32*k)
  - positives = diagonal of the block at columns 4096..5119, pulled from
    the raw-sim PSUM tiles with an identity-masked multiply-accumulate
  - exact diagonal e^2 subtracted; outputs per core:
    row partials [128,8], positives [128,8], column sums [6,512]
"""

import numpy as np
from contextlib import ExitStack

import ml_dtypes
import concourse.bass as bass
import concourse.tile as tile
from concourse import bacc, mybir
from concourse._compat import with_exitstack
from concourse.bass_utils import run_bass_kernel_spmd

B = 4096
D = 256
R = 2 * B
N_CORES = 8
SLAB = R // N_CORES          # 1024 rows per core
INV_T = 2.0
E2 = float(np.exp(2.0))

NCOL = 5 * SLAB              # 5120 columns computed per core
NG = 5                       # load groups of 1024 rows (8 tiles)
NJ = 10                      # rts groups (512 cols each)
M_TILES = SLAB // 128        # 8
GROUPS = [[0, 1, 2], [3, 4, 5], [6, 7], [8, 9]]   # chunk (512-col) groups
CS_CHUNKS = {0: [(2, 0)], 1: [(3, 0), (4, 1), (5, 2)], 2: [(6, 0), (7, 1)]}
N_ACC = 4                    # accum slots per m (one per group)

F32 = mybir.dt.float32
FP8 = mybir.dt.float8e4
NP_FP8 = ml_dtypes.float8_e4m3fn


@with_exitstack
def _loss_kernel(ctx: ExitStack, tc: "tile.TileContext", s1_ap: bass.AP,
                 pos_ap: bass.AP, cs_ap: bass.AP, reps_ap: bass.AP,
                 ident_ap: bass.AP, ones_ap: bass.AP, ident32_ap: bass.AP):
    nc = tc.nc
    mult = mybir.AluOpType.mult
    add = mybir.AluOpType.add
    bypass = mybir.AluOpType.bypass
    subtract = mybir.AluOpType.subtract
    lsr = mybir.AluOpType.logical_shift_right
    Exp = mybir.ActivationFunctionType.Exp
    DR = mybir.MatmulPerfMode.DoubleRow
    I32 = mybir.dt.int32

    xpool = ctx.enter_context(tc.tile_pool(name="x", bufs=NG))
    stats = ctx.enter_context(tc.tile_pool(name="stats", bufs=2))
    scales = ctx.enter_context(tc.tile_pool(name="scales", bufs=NG))
    zpool = ctx.enter_context(tc.tile_pool(name="z8", bufs=6))
    prodpool = ctx.enter_context(tc.tile_pool(name="prod", bufs=2))
    rtpool = ctx.enter_context(tc.tile_pool(name="repsT", bufs=NJ))
    cpool = ctx.enter_context(tc.tile_pool(name="const", bufs=1))
    epool = ctx.enter_context(tc.tile_pool(name="esc", bufs=8))
    accpool = ctx.enter_context(tc.tile_pool(name="acc", bufs=1))
    fpool = ctx.enter_context(tc.tile_pool(name="final", bufs=1))

    # PSUM: 1 bank transpose ping-pong + 2x3 banks matmul + 1 bank colsum
    tppool = ctx.enter_context(tc.tile_pool(name="tp", bufs=1, space="PSUM"))
    mmpool = ctx.enter_context(tc.tile_pool(name="mm", bufs=2, space="PSUM"))
    cspool = ctx.enter_context(tc.tile_pool(name="cs", bufs=1, space="PSUM"))

    ident = cpool.tile([128, 128], FP8, tag="ident")
    nc.sync.dma_start(ident[:], ident_ap[:])
    ones = cpool.tile([128, 2, 16], FP8, tag="ones")
    nc.sync.dma_start(ones[:], ones_ap[:])
    ident32 = cpool.tile([128, 128], F32, tag="ident32")
    nc.sync.dma_start(ident32[:], ident32_ap[:])
    magic = cpool.tile([128, 1], I32, tag="magic")
    nc.vector.memset(magic[:], 0x5F3759DF)

    # ---- loads (gpsimd SWDGE, striped over all 16 queues) ---------------
    xg = []
    for g in range(NG):
        xt = xpool.tile([128, 8, D], F32, tag="x", name=f"x{g}")
        src = reps_ap[g * 1024:(g + 1) * 1024, :].rearrange(
            "(t p) d -> p t d", p=128)
        nc.gpsimd.dma_start(xt[:], src)
        xg.append(xt)

    rts = [
        rtpool.tile([128, 2, 4, 128], FP8, tag="repsT", name=f"repsT{j}")
        for j in range(NJ)
    ]
    # fp8 PE transpose writes PSUM with element step 2 -> [..., 128, 2]
    tp = tppool.tile([128, 8, 128, 2], FP8, tag="tp")

    # ---- per-group: ssq -> scale -> normalize(fp8) -> transpose ---------
    scg = []

    def emit_group(g):
        ssq = stats.tile([128, 8], F32, tag="ssq", name=f"ssq{g}")
        for t in range(8):
            junk = prodpool.tile([128, D], F32, tag="prod", name=f"sq{g}_{t}")
            nc.vector.scalar_tensor_tensor(
                out=junk[:], in0=xg[g][:, t, :], scalar=1.0,
                in1=xg[g][:, t, :], op0=mult, op1=mult,
                accum_out=ssq[:, t:t + 1],
            )
        # rsqrt(ssq) on DVE: integer seed + 2 Newton iterations
        half = stats.tile([128, 8], I32, tag="half", name=f"half{g}")
        nc.vector.tensor_scalar(
            out=half[:], in0=ssq[:].bitcast(I32), scalar1=1, scalar2=None,
            op0=lsr,
        )
        y0 = stats.tile([128, 8], F32, tag="y0", name=f"y0{g}")
        nc.vector.scalar_tensor_tensor(
            out=y0[:].bitcast(I32), in0=magic[:].broadcast_to([128, 8]),
            scalar=0, in1=half[:], op0=bypass, op1=subtract,
        )
        yy = stats.tile([128, 8], F32, tag="yy", name=f"yy{g}")
        hh = stats.tile([128, 8], F32, tag="hh", name=f"hh{g}")
        y1 = stats.tile([128, 8], F32, tag="y1", name=f"y1{g}")
        sc = scales.tile([128, 8], F32, tag="scale", name=f"sc{g}")
        nc.vector.tensor_tensor(out=yy[:], in0=y0[:], in1=y0[:], op=mult)
        nc.vector.scalar_tensor_tensor(
            out=hh[:], in0=ssq[:], scalar=-0.5, in1=yy[:], op0=mult, op1=mult)
        nc.vector.scalar_tensor_tensor(
            out=y1[:], in0=hh[:], scalar=1.5, in1=y0[:], op0=add, op1=mult)
        nc.vector.tensor_tensor(out=yy[:], in0=y1[:], in1=y1[:], op=mult)
        nc.vector.scalar_tensor_tensor(
            out=hh[:], in0=ssq[:], scalar=-0.5, in1=yy[:], op0=mult, op1=mult)
        nc.vector.scalar_tensor_tensor(
            out=sc[:], in0=hh[:], scalar=1.5, in1=y1[:], op0=add, op1=mult)
        scg.append(sc)

        z8s = []
        for t in range(8):
            z8 = zpool.tile([128, D], FP8, tag="z8", name=f"z8_{g}_{t}")
            nc.vector.tensor_scalar(
                out=z8[:], in0=xg[g][:, t, :],
                scalar1=sc[:, t:t + 1], scalar2=None, op0=mult,
            )
            z8s.append(z8)
        # transpose the 8 tiles (2 rts groups) on the PE
        for jj in range(2):
            j = 2 * g + jj
            for k in range(2):
                for i in range(4):
                    nc.tensor.transpose(
                        tp[:, 4 * k + i, :, 0:1],
                        z8s[4 * jj + i][:, k * 128:(k + 1) * 128],
                        ident[:],
                    )
                nc.vector.tensor_copy(
                    rts[j][:, k, :, :], tp[:, 4 * k:4 * k + 4, :, 0:1])

    # ---- similarity group: DoubleRow matmuls + fused exp/rowsum ---------
    denacc = accpool.tile([128, M_TILES * N_ACC], F32, tag="denacc")
    posneg = accpool.tile([128, M_TILES], F32, tag="posneg")
    esc_live = {}

    def emit_mm_group(gi):
        chunks = GROUPS[gi]
        nch = len(chunks)
        for m in range(M_TILES):
            pt = mmpool.tile([128, 3, 512], F32, tag="mm", name=f"pt{gi}_{m}")
            lhsT = rts[m // 4][:, :, m % 4, :]
            for ci, c in enumerate(chunks):
                nc.tensor.matmul(
                    pt[:, ci, :], lhsT=lhsT, rhs=rts[c][:, :, :, :],
                    perf_mode=DR,
                )
            if m % 2 == 0:
                esc = epool.tile([128, 2, 3, 512], FP8, tag="esc",
                                 name=f"esc{gi}_{m // 2}")
                esc_live[(gi, m // 2)] = esc
            esc = esc_live[(gi, m // 2)]
            idx = m * N_ACC + gi
            nc.scalar.activation(
                esc[:, m % 2, :nch, :], pt[:, :nch, :], Exp, scale=INV_T,
                accum_out=denacc[:, idx:idx + 1],
            )
            if gi == 3:
                # positive pair sim = diag of the cols-4096..5119 block:
                # chunk 8 (ci 0) for m<4, chunk 9 (ci 1) for m>=4
                ci = 0 if m < 4 else 1
                off = (m % 4) * 128
                junk = prodpool.tile([128, 128], F32, tag="posj",
                                     name=f"posj{m}", bufs=2)
                nc.vector.scalar_tensor_tensor(
                    out=junk[:], in0=pt[:, ci, off:off + 128], scalar=1.0,
                    in1=ident32[:], op0=mult, op1=mult,
                    accum_out=posneg[:, m:m + 1],
                )

    def emit_colsums(gi):
        for c, slot in CS_CHUNKS[gi]:
            ci = c - GROUPS[gi][0]
            cs = cspool.tile([128, 512], F32, tag="cs", name=f"cs{gi}_{c}")
            for mp in range(4):
                nc.tensor.matmul(
                    cs[0:16, :],
                    lhsT=ones[:, :, :],
                    rhs=esc_live[(gi, mp)][:, :, ci, :],
                    perf_mode=DR,
                    start=(mp == 0), stop=(mp == 3),
                )
            csb = fpool.tile([128, 512], F32, tag="csb", name=f"csb{gi}_{c}",
                             bufs=2)
            nc.vector.tensor_copy(csb[0:1, :], cs[0:1, :])
            nc.sync.dma_start(cs_ap[c - 2:c - 1, :], csb[0:1, :])

    emit_group(0)
    emit_group(1)
    emit_mm_group(0)
    emit_group(2)
    emit_mm_group(1)
    emit_colsums(0)
    emit_group(3)
    emit_mm_group(2)
    emit_colsums(1)
    emit_group(4)

    emit_mm_group(3)
    nc.sync.dma_start(pos_ap[:], posneg[:])
    emit_colsums(2)

    # ---- finalize: reduce per-group row partials, subtract e^2 ----------
    drow = fpool.tile([128, M_TILES], F32, tag="drow")
    nc.vector.tensor_reduce(
        out=drow[:],
        in_=denacc[:].rearrange("p (m g) -> p m g", g=N_ACC),
        axis=mybir.AxisListType.X,
        op=add,
    )
    s1 = fpool.tile([128, M_TILES], F32, tag="s1")
    nc.vector.tensor_scalar(
        out=s1[:], in0=drow[:], scalar1=-E2, scalar2=None, op0=add,
    )
    nc.sync.dma_start(s1_ap[:], s1[:])


_CACHE = {}


def _get_compiled():
    if "nc" not in _CACHE:
        nc = bacc.Bacc("TRN2", target_bir_lowering=False, debug=False)
        reps_in = nc.dram_tensor("reps", [NCOL, D], F32, kind="ExternalInput")
        ident_t = nc.inline_tensor(np.eye(128, dtype=NP_FP8), name="ident")
        ones_t = nc.inline_tensor(np.ones((128, 2, 16), dtype=NP_FP8),
                                  name="ones")
        ident32_t = nc.inline_tensor(np.eye(128, dtype=np.float32),
                                     name="ident32")
        s1_out = nc.dram_tensor("s1", [128, M_TILES], F32,
                                kind="ExternalOutput")
        pos_out = nc.dram_tensor("pos", [128, M_TILES], F32,
                                 kind="ExternalOutput")
        cs_out = nc.dram_tensor("csum", [6, 512], F32, kind="ExternalOutput")
        with tile.TileContext(nc) as tc:
            _loss_kernel(tc, s1_out.ap(), pos_out.ap(), cs_out.ap(),
                         reps_in.ap(), ident_t.ap(), ones_t.ap(),
                         ident32_t.ap())
        nc.compile()
        _CACHE["nc"] = nc
    return _CACHE["nc"]


def make_in_maps(emb_i: np.ndarray, emb_j: np.ndarray):
    reps = np.concatenate(
        [np.asarray(emb_i, dtype=np.float32),
         np.asarray(emb_j, dtype=np.float32)],
        axis=0,
    )
    return [
        {"reps": np.ascontiguousarray(
            np.roll(reps, -c * SLAB, axis=0)[:NCOL])}
        for c in range(N_CORES)
    ]


def run_spmd(emb_i, emb_j, **kwargs):
    nc = _get_compiled()
    in_maps = make_in_maps(emb_i, emb_j)
    return run_bass_kernel_spmd(nc, in_maps, core_ids=list(range(N_CORES)),
                                **kwargs)


def combine(results) -> np.ndarray:
    """Host-side combine: assemble denominators from the per-core row
    partials + exchanged column sums, take logs, and reduce the loss."""
    s1 = [results[c]["s1"].astype(np.float64).T.reshape(SLAB)
          for c in range(N_CORES)]
    pos = [results[c]["pos"].astype(np.float64).T.reshape(SLAB)
           for c in range(N_CORES)]
    cs = [results[c]["csum"].astype(np.float64) for c in range(N_CORES)]
    total = 0.0
    for c in range(N_CORES):
        c1 = cs[(c + 7) % 8][0:2].reshape(SLAB)
        c2 = cs[(c + 6) % 8][2:4].reshape(SLAB)
        c3 = cs[(c + 5) % 8][4:6].reshape(SLAB)
        den = s1[c] + c1 + c2 + c3
        total += float(np.sum(np.log(den) - INV_T * pos[c]))
    return np.array(total / R, dtype=np.float32)


def kernel(emb_i: np.ndarray, emb_j: np.ndarray) -> np.ndarray:
    res = run_spmd(emb_i, emb_j)
    return combine(res.results)



# revision 17
# speedup vs baseline: 2.3266x; 1.0028x over previous
"""NT-Xent / SimCLR contrastive loss on 8 Trainium2 NeuronCores (v3).

Math (matches the jax reference):
    z = l2_normalize(concat([emb_i, emb_j]))          # [2B, D] unit rows
    sim = z @ z.T                                     # cosine similarities
    denom_r = sum_{j != r} exp(sim_rj / T)
    pos_r   = z_r . z_{(r+B) mod 2B}                  # the positive pair
    loss = mean_r( log(denom_r) - pos_r / T )

Sharding: the 2B=8192 rows are data-parallel over 8 cores. Each core
receives the representation matrix ROTATED by -c*1024 rows, so its slab
is always local rows 0..1023 (one SPMD program for all cores).

v3 exploits the symmetry of exp(sim/T): each core computes only local
columns 0..5119 (blocks 0..4 of 8). The row-sum contribution of the
missing blocks 5..7 equals COLUMN sums of blocks 3,2,1 on cores
c+5,c+6,c+7; those [1024]-vectors are tiny and exchanged on the host,
which assembles denominators and takes the log (the "all-reduce" step).

Per-core pipeline (v4):
  - 5 x 1MiB strided loads striped over the 16 SWDGE queues
  - per-group sum-of-squares (DVE square+row-accumulate), rsqrt via
    integer-seed + 2 Newton steps entirely on DVE (keeps ACT exp-only,
    one activation table set), normalize+cast to fp8e4 on DVE
  - transpose to [D, 5120] fp8 fully on the PE (fp8 identity, 1 cyc/row)
    through a single one-bank PSUM tile with ping-pong slots
  - similarity via fp8 DoubleRow matmuls: one [128x512] output per
    instruction contracts the full D=256 at 0.5 cycles/row
  - ACT computes exp(2*sim) out of PSUM with fused row accumulation;
    exp tiles for blocks 1..3 are kept in fp8 and column-summed with
    ones-DoubleRow matmuls into per-chunk PSUM slots (partition 32*k)
  - positives = diagonal of the block at columns 4096..5119, pulled from
    the raw-sim PSUM tiles with an identity-masked multiply-accumulate
  - exact diagonal e^2 subtracted; outputs per core:
    row partials [128,8], positives [128,8], column sums [6,512]
"""

import numpy as np
from contextlib import ExitStack

import ml_dtypes
import concourse.bass as bass
import concourse.tile as tile
from concourse import bacc, mybir
from concourse._compat import with_exitstack
from concourse.bass_utils import run_bass_kernel_spmd

B = 4096
D = 256
R = 2 * B
N_CORES = 8
SLAB = R // N_CORES          # 1024 rows per core
INV_T = 2.0
E2 = float(np.exp(2.0))

NCOL = 5 * SLAB              # 5120 columns computed per core
NG = 5                       # load groups of 1024 rows (8 tiles)
NJ = 10                      # rts groups (512 cols each)
M_TILES = SLAB // 128        # 8
GROUPS = [[0, 1, 2], [3, 4, 5], [6, 7], [8, 9]]   # chunk (512-col) groups
CS_CHUNKS = {0: [(2, 0)], 1: [(3, 0), (4, 1), (5, 2)], 2: [(6, 0), (7, 1)]}
N_ACC = 4                    # accum slots per m (one per group)

F32 = mybir.dt.float32
FP8 = mybir.dt.float8e4
NP_FP8 = ml_dtypes.float8_e4m3fn


@with_exitstack
def _loss_kernel(ctx: ExitStack, tc: "tile.TileContext", s1_ap: bass.AP,
                 pos_ap: bass.AP, cs_ap: bass.AP, reps_ap: bass.AP,
                 ident_ap: bass.AP, ones_ap: bass.AP, ident32_ap: bass.AP):
    nc = tc.nc
    mult = mybir.AluOpType.mult
    add = mybir.AluOpType.add
    bypass = mybir.AluOpType.bypass
    subtract = mybir.AluOpType.subtract
    lsr = mybir.AluOpType.logical_shift_right
    Exp = mybir.ActivationFunctionType.Exp
    DR = mybir.MatmulPerfMode.DoubleRow
    I32 = mybir.dt.int32

    xpool = ctx.enter_context(tc.tile_pool(name="x", bufs=NG))
    stats = ctx.enter_context(tc.tile_pool(name="stats", bufs=2))
    scales = ctx.enter_context(tc.tile_pool(name="scales", bufs=NG))
    zpool = ctx.enter_context(tc.tile_pool(name="z8", bufs=6))
    prodpool = ctx.enter_context(tc.tile_pool(name="prod", bufs=2))
    rtpool = ctx.enter_context(tc.tile_pool(name="repsT", bufs=NJ))
    cpool = ctx.enter_context(tc.tile_pool(name="const", bufs=1))
    epool = ctx.enter_context(tc.tile_pool(name="esc", bufs=8))
    accpool = ctx.enter_context(tc.tile_pool(name="acc", bufs=1))
    fpool = ctx.enter_context(tc.tile_pool(name="final", bufs=1))

    # PSUM: 1 bank transpose ping-pong + 2x3 banks matmul + 1 bank colsum
    tppool = ctx.enter_context(tc.tile_pool(name="tp", bufs=1, space="PSUM"))
    mmpool = ctx.enter_context(tc.tile_pool(name="mm", bufs=2, space="PSUM"))
    cspool = ctx.enter_context(tc.tile_pool(name="cs", bufs=1, space="PSUM"))

    # ---- loads first: 2 issues per group for finer completion grain -----
    xg = []
    for g in range(NG):
        xt = [xpool.tile([128, 4, D], F32, tag=f"x{h}", name=f"x{g}_{h}")
              for h in range(2)]
        for h in range(2):
            src = reps_ap[g * 1024 + h * 512:g * 1024 + (h + 1) * 512, :] \
                .rearrange("(t p) d -> p t d", p=128)
            nc.gpsimd.dma_start(xt[h][:], src)
        xg.append(xt)

    ident = cpool.tile([128, 128], FP8, tag="ident")
    nc.sync.dma_start(ident[:], ident_ap[:])
    ones = cpool.tile([128, 2, 16], FP8, tag="ones")
    nc.sync.dma_start(ones[:], ones_ap[:])
    ident32 = cpool.tile([128, 128], F32, tag="ident32")
    nc.sync.dma_start(ident32[:], ident32_ap[:])
    magic = cpool.tile([128, 1], I32, tag="magic")
    nc.vector.memset(magic[:], 0x5F3759DF)

    rts = [
        rtpool.tile([128, 2, 4, 128], FP8, tag="repsT", name=f"repsT{j}")
        for j in range(NJ)
    ]
    # fp8 PE transpose writes PSUM with element step 2 -> [..., 128, 2]
    tp = tppool.tile([128, 8, 128, 2], FP8, tag="tp")

    # ---- per-group: ssq -> scale -> normalize(fp8) -> transpose ---------
    scg = []

    def emit_group(g):
        ssq = stats.tile([128, 8], F32, tag="ssq", name=f"ssq{g}")
        for t in range(8):
            junk = prodpool.tile([128, D], F32, tag="prod", name=f"sq{g}_{t}")
            nc.vector.scalar_tensor_tensor(
                out=junk[:], in0=xg[g][t // 4][:, t % 4, :], scalar=1.0,
                in1=xg[g][t // 4][:, t % 4, :], op0=mult, op1=mult,
                accum_out=ssq[:, t:t + 1],
            )
        # rsqrt(ssq) on DVE: integer seed + 2 Newton iterations
        half = stats.tile([128, 8], I32, tag="half", name=f"half{g}")
        nc.vector.tensor_scalar(
            out=half[:], in0=ssq[:].bitcast(I32), scalar1=1, scalar2=None,
            op0=lsr,
        )
        y0 = stats.tile([128, 8], F32, tag="y0", name=f"y0{g}")
        nc.vector.scalar_tensor_tensor(
            out=y0[:].bitcast(I32), in0=magic[:].broadcast_to([128, 8]),
            scalar=0, in1=half[:], op0=bypass, op1=subtract,
        )
        yy = stats.tile([128, 8], F32, tag="yy", name=f"yy{g}")
        hh = stats.tile([128, 8], F32, tag="hh", name=f"hh{g}")
        y1 = stats.tile([128, 8], F32, tag="y1", name=f"y1{g}")
        sc = scales.tile([128, 8], F32, tag="scale", name=f"sc{g}")
        nc.vector.tensor_tensor(out=yy[:], in0=y0[:], in1=y0[:], op=mult)
        nc.vector.scalar_tensor_tensor(
            out=hh[:], in0=ssq[:], scalar=-0.5, in1=yy[:], op0=mult, op1=mult)
        nc.vector.scalar_tensor_tensor(
            out=y1[:], in0=hh[:], scalar=1.5, in1=y0[:], op0=add, op1=mult)
        nc.vector.tensor_tensor(out=yy[:], in0=y1[:], in1=y1[:], op=mult)
        nc.vector.scalar_tensor_tensor(
            out=hh[:], in0=ssq[:], scalar=-0.5, in1=yy[:], op0=mult, op1=mult)
        nc.vector.scalar_tensor_tensor(
            out=sc[:], in0=hh[:], scalar=1.5, in1=y1[:], op0=add, op1=mult)
        scg.append(sc)

        z8s = []
        for t in range(8):
            z8 = zpool.tile([128, D], FP8, tag="z8", name=f"z8_{g}_{t}")
            nc.vector.tensor_scalar(
                out=z8[:], in0=xg[g][t // 4][:, t % 4, :],
                scalar1=sc[:, t:t + 1], scalar2=None, op0=mult,
            )
            z8s.append(z8)
        # transpose the 8 tiles (2 rts groups) on the PE
        for jj in range(2):
            j = 2 * g + jj
            for k in range(2):
                for i in range(4):
                    nc.tensor.transpose(
                        tp[:, 4 * k + i, :, 0:1],
                        z8s[4 * jj + i][:, k * 128:(k + 1) * 128],
                        ident[:],
                    )
                nc.vector.tensor_copy(
                    rts[j][:, k, :, :], tp[:, 4 * k:4 * k + 4, :, 0:1])

    # ---- similarity group: DoubleRow matmuls + fused exp/rowsum ---------
    denacc = accpool.tile([128, M_TILES * N_ACC], F32, tag="denacc")
    posneg = accpool.tile([128, M_TILES], F32, tag="posneg")
    esc_live = {}

    def emit_mm_group(gi):
        chunks = GROUPS[gi]
        nch = len(chunks)
        for m in range(M_TILES):
            pt = mmpool.tile([128, 3, 512], F32, tag="mm", name=f"pt{gi}_{m}")
            lhsT = rts[m // 4][:, :, m % 4, :]
            for ci, c in enumerate(chunks):
                nc.tensor.matmul(
                    pt[:, ci, :], lhsT=lhsT, rhs=rts[c][:, :, :, :],
                    perf_mode=DR,
                )
            if m % 2 == 0:
                esc = epool.tile([128, 2, 3, 512], FP8, tag="esc",
                                 name=f"esc{gi}_{m // 2}")
                esc_live[(gi, m // 2)] = esc
            esc = esc_live[(gi, m // 2)]
            idx = m * N_ACC + gi
            nc.scalar.activation(
                esc[:, m % 2, :nch, :], pt[:, :nch, :], Exp, scale=INV_T,
                accum_out=denacc[:, idx:idx + 1],
            )
            if gi == 3:
                # positive pair sim = diag of the cols-4096..5119 block:
                # chunk 8 (ci 0) for m<4, chunk 9 (ci 1) for m>=4
                ci = 0 if m < 4 else 1
                off = (m % 4) * 128
                junk = prodpool.tile([128, 128], F32, tag="posj",
                                     name=f"posj{m}", bufs=2)
                nc.vector.scalar_tensor_tensor(
                    out=junk[:], in0=pt[:, ci, off:off + 128], scalar=1.0,
                    in1=ident32[:], op0=mult, op1=mult,
                    accum_out=posneg[:, m:m + 1],
                )

    def emit_colsums(gi):
        for c, slot in CS_CHUNKS[gi]:
            ci = c - GROUPS[gi][0]
            cs = cspool.tile([128, 512], F32, tag="cs", name=f"cs{gi}_{c}")
            for mp in range(4):
                nc.tensor.matmul(
                    cs[0:16, :],
                    lhsT=ones[:, :, :],
                    rhs=esc_live[(gi, mp)][:, :, ci, :],
                    perf_mode=DR,
                    start=(mp == 0), stop=(mp == 3),
                )
            csb = fpool.tile([128, 512], F32, tag="csb", name=f"csb{gi}_{c}",
                             bufs=2)
            nc.vector.tensor_copy(csb[0:1, :], cs[0:1, :])
            nc.sync.dma_start(cs_ap[c - 2:c - 1, :], csb[0:1, :])

    emit_group(0)
    emit_group(1)
    emit_mm_group(0)
    emit_group(2)
    emit_mm_group(1)
    emit_colsums(0)
    emit_group(3)
    emit_mm_group(2)
    emit_colsums(1)
    emit_group(4)

    emit_mm_group(3)
    nc.sync.dma_start(pos_ap[:], posneg[:])
    emit_colsums(2)

    # ---- finalize: reduce per-group row partials, subtract e^2 ----------
    drow = fpool.tile([128, M_TILES], F32, tag="drow")
    nc.vector.tensor_reduce(
        out=drow[:],
        in_=denacc[:].rearrange("p (m g) -> p m g", g=N_ACC),
        axis=mybir.AxisListType.X,
        op=add,
    )
    s1 = fpool.tile([128, M_TILES], F32, tag="s1")
    nc.vector.tensor_scalar(
        out=s1[:], in0=drow[:], scalar1=-E2, scalar2=None, op0=add,
    )
    nc.sync.dma_start(s1_ap[:], s1[:])


_CACHE = {}


def _get_compiled():
    if "nc" not in _CACHE:
        nc = bacc.Bacc("TRN2", target_bir_lowering=False, debug=False)
        reps_in = nc.dram_tensor("reps", [NCOL, D], F32, kind="ExternalInput")
        ident_t = nc.inline_tensor(np.eye(128, dtype=NP_FP8), name="ident")
        ones_t = nc.inline_tensor(np.ones((128, 2, 16), dtype=NP_FP8),
                                  name="ones")
        ident32_t = nc.inline_tensor(np.eye(128, dtype=np.float32),
                                     name="ident32")
        s1_out = nc.dram_tensor("s1", [128, M_TILES], F32,
                                kind="ExternalOutput")
        pos_out = nc.dram_tensor("pos", [128, M_TILES], F32,
                                 kind="ExternalOutput")
        cs_out = nc.dram_tensor("csum", [6, 512], F32, kind="ExternalOutput")
        with tile.TileContext(nc) as tc:
            _loss_kernel(tc, s1_out.ap(), pos_out.ap(), cs_out.ap(),
                         reps_in.ap(), ident_t.ap(), ones_t.ap(),
                         ident32_t.ap())
        nc.compile()
        _CACHE["nc"] = nc
    return _CACHE["nc"]


def make_in_maps(emb_i: np.ndarray, emb_j: np.ndarray):
    reps = np.concatenate(
        [np.asarray(emb_i, dtype=np.float32),
         np.asarray(emb_j, dtype=np.float32)],
        axis=0,
    )
    return [
        {"reps": np.ascontiguousarray(
            np.roll(reps, -c * SLAB, axis=0)[:NCOL])}
        for c in range(N_CORES)
    ]


def run_spmd(emb_i, emb_j, **kwargs):
    nc = _get_compiled()
    in_maps = make_in_maps(emb_i, emb_j)
    return run_bass_kernel_spmd(nc, in_maps, core_ids=list(range(N_CORES)),
                                **kwargs)


def combine(results) -> np.ndarray:
    """Host-side combine: assemble denominators from the per-core row
    partials + exchanged column sums, take logs, and reduce the loss."""
    s1 = [results[c]["s1"].astype(np.float64).T.reshape(SLAB)
          for c in range(N_CORES)]
    pos = [results[c]["pos"].astype(np.float64).T.reshape(SLAB)
           for c in range(N_CORES)]
    cs = [results[c]["csum"].astype(np.float64) for c in range(N_CORES)]
    total = 0.0
    for c in range(N_CORES):
        c1 = cs[(c + 7) % 8][0:2].reshape(SLAB)
        c2 = cs[(c + 6) % 8][2:4].reshape(SLAB)
        c3 = cs[(c + 5) % 8][4:6].reshape(SLAB)
        den = s1[c] + c1 + c2 + c3
        total += float(np.sum(np.log(den) - INV_T * pos[c]))
    return np.array(total / R, dtype=np.float32)


def kernel(emb_i: np.ndarray, emb_j: np.ndarray) -> np.ndarray:
    res = run_spmd(emb_i, emb_j)
    return combine(res.results)

